# revision 3
# baseline (speedup 1.0000x reference)
"""MHA kernel for Trainium2, 8-way sharded (batch x head-group).

Reference: out = softmax((q@Wq+bq)(k@Wk+bk)^T / sqrt(64)) (v@Wv+bv) @ Wo + bo
Shapes: q,k,v [2, 2048, 768]; 12 heads x 64 dim.

Sharding (Megatron column-parallel): core c in 0..7 -> batch b = c//4,
head group g = c%4 (3 heads = channel slice 192g:192(g+1)). Each core
computes its heads' projections, attention, and partial out-proj
(Wo rows for its heads). Host sums the 4 partials per batch (+bo).

Device layout notes:
- Host pre-transposes q/k/v to [768, 2048] (bf16) so projections can
  contract over the partition dim without on-device transposes.
- Scores are computed transposed: S^T[k, q] = Kh^T.T @ Qh^T, so exp
  output P^T[k, q] feeds PV as the moving operand with lhsT = [Vh | 1]
  (the appended ones column yields the softmax row-sums for free).
- Softmax skips max-subtraction: scores ~ N(0,1), no overflow in fp32.
- Normalization: r = 1/l broadcast across partitions via a K=1 matmul
  (ones[1,64].T @ r[1,q]), then DVE multiply during PSUM evacuation.
"""

import sys

if "/opt/trn_rl_repo" not in sys.path:
    sys.path.insert(0, "/opt/trn_rl_repo")

import numpy as np
import ml_dtypes

S = 2048
D = 768
DH = 64
HG = 3          # heads per core
CS = HG * DH    # 192 channel slice per core
NCORES = 8
ECH = D // 128  # 6 contraction chunks

_cached = {}


def _build_nc():
    import concourse.bass as bass
    from concourse import bacc
    import concourse.mybir as mybir
    import concourse.tile as tile

    f32 = mybir.dt.float32
    bf16 = mybir.dt.bfloat16
    FP = mybir.dt.float32  # on-chip compute dtype

    nc = bacc.Bacc(None, target_bir_lowering=False)

    xq_d = nc.dram_tensor("xqT", [D, S], bf16, kind="ExternalInput")
    xk_d = nc.dram_tensor("xkT", [D, S], bf16, kind="ExternalInput")
    xv_d = nc.dram_tensor("xvT", [D, S], bf16, kind="ExternalInput")
    wq_d = nc.dram_tensor("wq", [D, CS], bf16, kind="ExternalInput")
    wk_d = nc.dram_tensor("wk", [D, CS], bf16, kind="ExternalInput")
    wv_d = nc.dram_tensor("wv", [D, CS], bf16, kind="ExternalInput")
    wo_d = nc.dram_tensor("wo", [CS, D], f32, kind="ExternalInput")
    bq_d = nc.dram_tensor("bq", [CS, 1], f32, kind="ExternalInput")
    bk_d = nc.dram_tensor("bk", [CS, 1], f32, kind="ExternalInput")
    bv_d = nc.dram_tensor("bv", [128, CS], f32, kind="ExternalInput")
    out_d = nc.dram_tensor("out", [S, D], f32, kind="ExternalOutput")

    Exp = mybir.ActivationFunctionType.Exp
    PSUM = bass.MemorySpace.PSUM

    with tile.TileContext(nc) as tc:
        with (
            tc.tile_pool(name="cst", bufs=1) as cst,
            tc.tile_pool(name="big", bufs=1) as bigp,
            tc.tile_pool(name="pt", bufs=2) as ptp,
            tc.tile_pool(name="rr", bufs=2) as rrp,
            tc.tile_pool(name="osb", bufs=2) as osbp,
            tc.tile_pool(name="psA", bufs=1, space=PSUM) as psA,
            tc.tile_pool(name="psB", bufs=2, space=PSUM) as psB,
        ):
            # ---- constants / weights ----
            wq_sb = cst.tile([128, ECH, CS], bf16, tag="wq")
            nc.sync.dma_start(wq_sb[:], xq_rearr(wq_d))
            wk_sb = cst.tile([128, ECH, CS], bf16, tag="wk")
            nc.sync.dma_start(wk_sb[:], xq_rearr(wk_d))
            wv_sb = cst.tile([128, ECH, CS], bf16, tag="wv")
            nc.sync.dma_start(wv_sb[:], xq_rearr(wv_d))

            wo_sb = []
            for h in range(HG):
                t = cst.tile([DH, D], f32, tag=f"wo{h}")
                nc.sync.dma_start(t[:], wo_d[h * DH:(h + 1) * DH, :])
                wo_sb.append(t)

            bq_a = cst.tile([128, 1], f32, tag="bqa")
            nc.sync.dma_start(bq_a[:], bq_d[0:128, :])
            bq_b = cst.tile([DH, 1], f32, tag="bqb")
            nc.sync.dma_start(bq_b[:], bq_d[128:CS, :])
            bk_a = cst.tile([128, 1], f32, tag="bka")
            nc.sync.dma_start(bk_a[:], bk_d[0:128, :])
            bk_b = cst.tile([DH, 1], f32, tag="bkb")
            nc.sync.dma_start(bk_b[:], bk_d[128:CS, :])
            bv_sb = cst.tile([128, CS], f32, tag="bv")
            nc.sync.dma_start(bv_sb[:], bv_d[:])

            ones_c = cst.tile([1, DH], f32, tag="ones")
            nc.vector.memset(ones_c[:], 1.0)

            # ---- inputs ----
            xq_sb = bigp.tile([128, ECH, S], bf16, tag="xq")
            nc.sync.dma_start(xq_sb[:], xq_rearr(xq_d))
            xk_sb = bigp.tile([128, ECH, S], bf16, tag="xk")
            nc.sync.dma_start(xk_sb[:], xq_rearr(xk_d))
            xv_sb = bigp.tile([128, ECH, S], bf16, tag="xv")
            nc.sync.dma_start(xv_sb[:], xq_rearr(xv_d))

            # ---- projections ----
            # Q/K transposed per-head layout: heads 0,1 stacked [128, S]; head2 [64, S]
            qhT_a = bigp.tile([128, S], FP, tag="qa")
            qhT_b = bigp.tile([DH, S], FP, tag="qb")
            khT_a = bigp.tile([128, S], FP, tag="ka")
            khT_b = bigp.tile([DH, S], FP, tag="kb")

            for x_sb, w_sb, b_a, b_b, o_a, o_b in (
                (xq_sb, wq_sb, bq_a, bq_b, qhT_a, qhT_b),
                (xk_sb, wk_sb, bk_a, bk_b, khT_a, khT_b),
            ):
                for mc0, mw, bias, dest in ((0, 128, b_a, o_a), (128, DH, b_b, o_b)):
                    for sc in range(4):
                        ps = psB.tile([mw, 512], f32, tag="acc")
                        for e in range(ECH):
                            nc.tensor.matmul(
                                ps[:],
                                w_sb[:, e, mc0:mc0 + mw],
                                x_sb[:, e, sc * 512:(sc + 1) * 512],
                                start=(e == 0),
                                stop=(e == ECH - 1),
                            )
                        nc.vector.tensor_scalar_add(
                            dest[:, sc * 512:(sc + 1) * 512], ps[:], bias[:]
                        )

            # V natural layout [s, 3, 65] with ones in column 64
            vh = bigp.tile([128, 16, HG, DH + 1], FP, tag="vh")
            nc.vector.memset(vh[:, :, :, DH:DH + 1], 1.0)
            for sb in range(16):
                ps = psB.tile([128, CS], f32, tag="acc")
                for e in range(ECH):
                    nc.tensor.matmul(
                        ps[:],
                        xv_sb[:, e, sb * 128:(sb + 1) * 128],
                        wv_sb[:, e, :],
                        start=(e == 0),
                        stop=(e == ECH - 1),
                    )
                nc.vector.tensor_add(
                    vh[:, sb, :, 0:DH],
                    ps[:].rearrange("p (h d) -> p h d", h=HG),
                    bv_sb[:].rearrange("p (h d) -> p h d", h=HG),
                )

            # ---- attention ----
            ohT = []
            for h in range(HG):
                ohT.append(bigp.tile([DH, S], FP, tag=f"oh{h}", name=f"oh{h}"))

            for h in range(HG):
                if h < 2:
                    qh = qhT_a[h * DH:(h + 1) * DH, :]
                    kh = khT_a[h * DH:(h + 1) * DH, :]
                else:
                    qh = qhT_b[:, :]
                    kh = khT_b[:, :]
                for qb in range(2):  # q blocks of 1024
                    q0 = qb * 1024
                    po = psB.tile([DH + 1, 1024], f32, tag="acc")
                    for kp in range(8):  # pairs of k chunks
                        ps = psA.tile([128, 2, 1024], f32, tag="sc")
                        for j in range(2):
                            kc = kp * 2 + j
                            for nh in range(2):
                                nc.tensor.matmul(
                                    ps[:, j, nh * 512:(nh + 1) * 512],
                                    kh[:, kc * 128:(kc + 1) * 128],
                                    qh[:, q0 + nh * 512:q0 + (nh + 1) * 512],
                                )
                        pt = ptp.tile([128, 2, 1024], FP, tag="pt")
                        nc.scalar.activation(pt[:], ps[:], Exp, scale=0.125)
                        for j in range(2):
                            kc = kp * 2 + j
                            for nh in range(2):
                                nc.tensor.matmul(
                                    po[:, nh * 512:(nh + 1) * 512],
                                    vh[:, kc, h, :],
                                    pt[:, j, nh * 512:(nh + 1) * 512],
                                    start=(kc == 0),
                                    stop=(kc == 15),
                                )
                    # normalize: r = 1/l ; R = ones^T @ r ; ohT = po * R
                    r_sb = rrp.tile([1, 1024], f32, tag="r")
                    nc.vector.reciprocal(r_sb[:], po[DH:DH + 1, :])
                    R_ps = psB.tile([DH, 1024], f32, tag="acc")
                    for nh in range(2):
                        nc.tensor.matmul(
                            R_ps[:, nh * 512:(nh + 1) * 512],
                            ones_c[:],
                            r_sb[:, nh * 512:(nh + 1) * 512],
                        )
                    R_sb = rrp.tile([DH, 1024], f32, tag="R")
                    nc.vector.tensor_copy(R_sb[:], R_ps[:])
                    nc.vector.tensor_mul(
                        ohT[h][:, q0:q0 + 1024], po[0:DH, :], R_sb[:]
                    )

            # ---- out projection (partial; host adds bo and reduces) ----
            for qblk in range(16):
                o_sb = osbp.tile([128, D], f32, tag="osb")
                for half in range(2):
                    ps = psB.tile([128, 384], f32, tag="acc")
                    for h in range(HG):
                        nc.tensor.matmul(
                            ps[:],
                            ohT[h][:, qblk * 128:(qblk + 1) * 128],
                            wo_sb[h][:, half * 384:(half + 1) * 384],
                            start=(h == 0),
                            stop=(h == HG - 1),
                        )
                    nc.vector.tensor_copy(o_sb[:, half * 384:(half + 1) * 384], ps[:])
                nc.sync.dma_start(out_d[qblk * 128:(qblk + 1) * 128, :], o_sb[:])

    nc.compile()
    return nc


def xq_rearr(d):
    # [C*128, N] dram -> [128, C, N] (chunk-major partition layout)
    return d[:].rearrange("(c p) n -> p c n", p=128)


def _get_nc():
    if "nc" not in _cached:
        _cached["nc"] = _build_nc()
    return _cached["nc"]


def kernel(q, k, v, Wq, bq, Wk, bk, Wv, bv, Wo, bo):
    from concourse.bass_utils import run_bass_kernel_spmd

    bf16 = ml_dtypes.bfloat16
    q = np.asarray(q, np.float32)
    k = np.asarray(k, np.float32)
    v = np.asarray(v, np.float32)

    xqT = [np.ascontiguousarray(q[b].T).astype(bf16) for b in range(2)]
    xkT = [np.ascontiguousarray(k[b].T).astype(bf16) for b in range(2)]
    xvT = [np.ascontiguousarray(v[b].T).astype(bf16) for b in range(2)]

    in_maps = []
    for c in range(NCORES):
        b, g = divmod(c, 4)
        cs = slice(CS * g, CS * (g + 1))
        in_maps.append({
            "xqT": xqT[b],
            "xkT": xkT[b],
            "xvT": xvT[b],
            "wq": np.ascontiguousarray(Wq[:, cs]).astype(bf16),
            "wk": np.ascontiguousarray(Wk[:, cs]).astype(bf16),
            "wv": np.ascontiguousarray(Wv[:, cs]).astype(bf16),
            "wo": np.ascontiguousarray(Wo[cs, :]).astype(np.float32),
            "bq": np.asarray(bq[cs], np.float32).reshape(CS, 1),
            "bk": np.asarray(bk[cs], np.float32).reshape(CS, 1),
            "bv": np.tile(np.asarray(bv[cs], np.float32), (128, 1)),
        })

    nc = _get_nc()
    res = run_bass_kernel_spmd(
        nc, in_maps, core_ids=list(range(NCORES)), **_cached.get("run_kwargs", {})
    )
    _cached["last_results"] = res

    out = np.zeros((2, S, D), np.float32)
    for c in range(NCORES):
        b = c // 4
        out[b] += res.results[c]["out"]
    out += np.asarray(bo, np.float32)
    return out


# revision 11
# speedup vs baseline: 13266.4701x; 13266.4701x over previous
"""MHA kernel for Trainium2, 8-way sharded (batch x head-group).

Reference: out = softmax((q@Wq+bq)(k@Wk+bk)^T / sqrt(64)) (v@Wv+bv) @ Wo + bo
Shapes: q,k,v [2, 2048, 768]; 12 heads x 64 dim.

Sharding (Megatron column-parallel): core c in 0..7 -> batch b = c//4,
head group g = c%4 (3 heads = channel slice 192g:192(g+1)). Each core
computes its heads' projections, attention, and partial out-proj
(Wo rows for its heads). Host sums the 4 partials per batch (+bo).

Device layout notes:
- Host pre-transposes q/k/v to [768, 2048] (bf16) so projections can
  contract over the partition dim without on-device transposes.
- Scores are computed transposed: S^T[k, q] = Kh^T.T @ Qh^T, so exp
  output P^T[k, q] feeds PV as the moving operand with lhsT = [Vh | 1]
  (the appended ones column yields the softmax row-sums for free).
- Softmax skips max-subtraction: scores ~ N(0,1), no overflow in fp32.
- Normalization: r = 1/l broadcast across partitions via a K=1 matmul
  (ones[1,64].T @ r[1,q]), then DVE multiply during PSUM evacuation.
"""

import sys

if "/opt/trn_rl_repo" not in sys.path:
    sys.path.insert(0, "/opt/trn_rl_repo")

import numpy as np
import ml_dtypes

S = 2048
D = 768
DH = 64
HG = 3          # heads per core
CS = HG * DH    # 192 channel slice per core
NCORES = 8
ECH = D // 128  # 6 contraction chunks

_cached = {}


def _build_nc():
    import concourse.bass as bass
    from concourse import bacc
    import concourse.mybir as mybir
    import concourse.tile as tile

    f32 = mybir.dt.float32
    f32r = mybir.dt.float32r
    bf16 = mybir.dt.bfloat16
    FP = mybir.dt.float32  # on-chip compute dtype

    nc = bacc.Bacc(None, target_bir_lowering=False)

    xq_d = nc.dram_tensor("xqT", [D, S], bf16, kind="ExternalInput")
    xk_d = nc.dram_tensor("xkT", [D, S], bf16, kind="ExternalInput")
    xv_d = nc.dram_tensor("xvT", [D, S], bf16, kind="ExternalInput")
    wq_d = nc.dram_tensor("wq", [D, CS], bf16, kind="ExternalInput")
    wk_d = nc.dram_tensor("wk", [D, CS], bf16, kind="ExternalInput")
    wv_d = nc.dram_tensor("wv", [D, CS], bf16, kind="ExternalInput")
    wo_d = nc.dram_tensor("wo", [CS, D], f32r, kind="ExternalInput")
    bq_d = nc.dram_tensor("bq", [CS, 1], f32, kind="ExternalInput")
    bk_d = nc.dram_tensor("bk", [CS, 1], f32, kind="ExternalInput")
    bv_d = nc.dram_tensor("bv", [128, CS], f32, kind="ExternalInput")
    out_d = nc.dram_tensor("out", [S, D], f32, kind="ExternalOutput")

    Exp = mybir.ActivationFunctionType.Exp
    PSUM = bass.MemorySpace.PSUM

    with tile.TileContext(nc) as tc:
        with (
            tc.tile_pool(name="cst", bufs=1) as cst,
            tc.tile_pool(name="big", bufs=1) as bigp,
            tc.tile_pool(name="pt", bufs=3) as ptp,
            tc.tile_pool(name="rr", bufs=2) as rrp,
            tc.tile_pool(name="osb", bufs=2) as osbp,
            tc.tile_pool(name="psA", bufs=2, space=PSUM) as psA,
            tc.tile_pool(name="psB", bufs=2, space=PSUM) as psB,
        ):
            # ---- constants / weights ----
            wq_sb = cst.tile([128, ECH, CS], bf16, tag="wq")
            nc.sync.dma_start(wq_sb[:], xq_rearr(wq_d))
            wk_sb = cst.tile([128, ECH, CS], bf16, tag="wk")
            nc.sync.dma_start(wk_sb[:], xq_rearr(wk_d))
            wv_sb = cst.tile([128, ECH, CS], bf16, tag="wv")
            nc.sync.dma_start(wv_sb[:], xq_rearr(wv_d))

            wo_sb = []
            for h in range(HG):
                t = cst.tile([DH, D], f32r, tag=f"wo{h}")
                nc.sync.dma_start(t[:], wo_d[h * DH:(h + 1) * DH, :])
                wo_sb.append(t)

            bq_a = cst.tile([128, 1], f32, tag="bqa")
            nc.sync.dma_start(bq_a[:], bq_d[0:128, :])
            bq_b = cst.tile([DH, 1], f32, tag="bqb")
            nc.sync.dma_start(bq_b[:], bq_d[128:CS, :])
            bk_a = cst.tile([128, 1], f32, tag="bka")
            nc.sync.dma_start(bk_a[:], bk_d[0:128, :])
            bk_b = cst.tile([DH, 1], f32, tag="bkb")
            nc.sync.dma_start(bk_b[:], bk_d[128:CS, :])
            bv_sb = cst.tile([128, CS], f32, tag="bv")
            nc.sync.dma_start(bv_sb[:], bv_d[:])

            ones_f = cst.tile([1, DH], f32, tag="onesf")
            nc.vector.memset(ones_f[:], 1.0)
            ones_c = cst.tile([1, DH], f32r, tag="ones")
            nc.vector.tensor_copy(ones_c[:], ones_f[:])
            onecol_f = cst.tile([128, HG, 1], f32, tag="onecf")
            nc.vector.memset(onecol_f[:], 1.0)
            onecol_r = cst.tile([128, HG, 1], f32r, tag="onecr")
            nc.vector.tensor_copy(onecol_r[:], onecol_f[:])

            # ---- inputs (per-chunk DMAs so projections start early) ----
            xq_sb = bigp.tile([128, ECH, S], bf16, tag="xq")
            xk_sb = bigp.tile([128, ECH, S], bf16, tag="xk")
            xv_sb = bigp.tile([128, ECH, S], bf16, tag="xv")
            for e in range(ECH):
                nc.sync.dma_start(xq_sb[:, e, :], xq_d[e * 128:(e + 1) * 128, :])
                nc.sync.dma_start(xk_sb[:, e, :], xk_d[e * 128:(e + 1) * 128, :])
                nc.sync.dma_start(xv_sb[:, e, :], xv_d[e * 128:(e + 1) * 128, :])

            # ---- projections ----
            # Q/K transposed per-head layout: heads 0,1 stacked [128, S]; head2 [64, S]
            qhT_a = bigp.tile([128, S], f32r, tag="qa")
            qhT_b = bigp.tile([DH, S], f32r, tag="qb")
            khT_a = bigp.tile([128, S], f32r, tag="ka")
            khT_b = bigp.tile([DH, S], f32r, tag="kb")

            for x_sb, w_sb, b_a, b_b, o_a, o_b in (
                (xq_sb, wq_sb, bq_a, bq_b, qhT_a, qhT_b),
                (xk_sb, wk_sb, bk_a, bk_b, khT_a, khT_b),
            ):
                for mc0, mw, bias, dest in ((0, 128, b_a, o_a), (128, DH, b_b, o_b)):
                    for sc in range(4):
                        ps = psB.tile([mw, 512], f32, tag="acc")
                        for e in range(ECH):
                            nc.tensor.matmul(
                                ps[:],
                                w_sb[:, e, mc0:mc0 + mw],
                                x_sb[:, e, sc * 512:(sc + 1) * 512],
                                start=(e == 0),
                                stop=(e == ECH - 1),
                            )
                        nc.vector.tensor_scalar_add(
                            dest[:, sc * 512:(sc + 1) * 512], ps[:], bias[:]
                        )

            # V natural layout [s, 3, 65] with ones in column 64
            vh = bigp.tile([128, 16, HG, DH + 1], f32r, tag="vh")
            for sb in range(16):
                ps = psB.tile([128, CS], f32, tag="acc")
                for e in range(ECH):
                    nc.tensor.matmul(
                        ps[:],
                        xv_sb[:, e, sb * 128:(sb + 1) * 128],
                        wv_sb[:, e, :],
                        start=(e == 0),
                        stop=(e == ECH - 1),
                    )
                nc.vector.tensor_copy(vh[:, sb, :, DH:DH + 1], onecol_r[:])
                nc.vector.tensor_add(
                    vh[:, sb, :, 0:DH],
                    ps[:].rearrange("p (h d) -> p h d", h=HG),
                    bv_sb[:].rearrange("p (h d) -> p h d", h=HG),
                )

            # ---- attention ----
            ohT = []
            for h in range(HG):
                ohT.append(bigp.tile([DH, S], f32r, tag=f"oh{h}", name=f"oh{h}"))

            for h in range(HG):
                if h < 2:
                    qh = qhT_a[h * DH:(h + 1) * DH, :]
                    kh = khT_a[h * DH:(h + 1) * DH, :]
                else:
                    qh = qhT_b[:, :]
                    kh = khT_b[:, :]
                for qb in range(2):  # q blocks of 1024
                    q0 = qb * 1024
                    po = psB.tile([DH + 1, 1024], f32, tag="acc")
                    for kc in range(16):  # k chunks of 128
                        ps = psA.tile([128, 1024], f32, tag="sc")
                        for nh in range(2):
                            nc.tensor.matmul(
                                ps[:, nh * 512:(nh + 1) * 512],
                                kh[:, kc * 128:(kc + 1) * 128],
                                qh[:, q0 + nh * 512:q0 + (nh + 1) * 512],
                            )
                        pt = ptp.tile([128, 1024], f32r, tag="pt")
                        nc.scalar.activation(pt[:], ps[:], Exp, scale=0.125)
                        for nh in range(2):
                            nc.tensor.matmul(
                                po[:, nh * 512:(nh + 1) * 512],
                                vh[:, kc, h, :],
                                pt[:, nh * 512:(nh + 1) * 512],
                                start=(kc == 0),
                                stop=(kc == 15),
                            )
                    # normalize: r = 1/l ; R = ones^T @ r ; ohT = po * R
                    r_sb = rrp.tile([1, 1024], f32r, tag="r")
                    with nc.allow_low_precision(reason="softmax denom in f32r"):
                        nc.vector.reciprocal(r_sb[:], po[DH:DH + 1, :])
                    R_ps = psB.tile([DH, 1024], f32, tag="acc")
                    for nh in range(2):
                        nc.tensor.matmul(
                            R_ps[:, nh * 512:(nh + 1) * 512],
                            ones_c[:],
                            r_sb[:, nh * 512:(nh + 1) * 512],
                        )
                    R_sb = rrp.tile([DH, 1024], f32, tag="R")
                    nc.vector.tensor_copy(R_sb[:], R_ps[:])
                    nc.vector.tensor_mul(
                        ohT[h][:, q0:q0 + 1024], po[0:DH, :], R_sb[:]
                    )

            # ---- out projection (partial; host adds bo and reduces) ----
            for qblk in range(16):
                o_sb = osbp.tile([128, D], f32, tag="osb")
                for half in range(2):
                    ps = psB.tile([128, 384], f32, tag="acc")
                    for h in range(HG):
                        nc.tensor.matmul(
                            ps[:],
                            ohT[h][:, qblk * 128:(qblk + 1) * 128],
                            wo_sb[h][:, half * 384:(half + 1) * 384],
                            start=(h == 0),
                            stop=(h == HG - 1),
                        )
                    nc.vector.tensor_copy(o_sb[:, half * 384:(half + 1) * 384], ps[:])
                nc.sync.dma_start(out_d[qblk * 128:(qblk + 1) * 128, :], o_sb[:])

    nc.compile()
    return nc


def xq_rearr(d):
    # [C*128, N] dram -> [128, C, N] (chunk-major partition layout)
    return d[:].rearrange("(c p) n -> p c n", p=128)


def _get_nc():
    if "nc" not in _cached:
        _cached["nc"] = _build_nc()
    return _cached["nc"]


def kernel(q, k, v, Wq, bq, Wk, bk, Wv, bv, Wo, bo):
    from concourse.bass_utils import run_bass_kernel_spmd

    bf16 = ml_dtypes.bfloat16
    q = np.asarray(q, np.float32)
    k = np.asarray(k, np.float32)
    v = np.asarray(v, np.float32)

    xqT = [np.ascontiguousarray(q[b].T).astype(bf16) for b in range(2)]
    xkT = [np.ascontiguousarray(k[b].T).astype(bf16) for b in range(2)]
    xvT = [np.ascontiguousarray(v[b].T).astype(bf16) for b in range(2)]

    in_maps = []
    for c in range(NCORES):
        b, g = divmod(c, 4)
        cs = slice(CS * g, CS * (g + 1))
        in_maps.append({
            "xqT": xqT[b],
            "xkT": xkT[b],
            "xvT": xvT[b],
            "wq": np.ascontiguousarray(Wq[:, cs]).astype(bf16),
            "wk": np.ascontiguousarray(Wk[:, cs]).astype(bf16),
            "wv": np.ascontiguousarray(Wv[:, cs]).astype(bf16),
            "wo": np.ascontiguousarray(Wo[cs, :]).astype(np.float32),
            "bq": np.asarray(bq[cs], np.float32).reshape(CS, 1),
            "bk": np.asarray(bk[cs], np.float32).reshape(CS, 1),
            "bv": np.tile(np.asarray(bv[cs], np.float32), (128, 1)),
        })

    nc = _get_nc()
    res = run_bass_kernel_spmd(
        nc, in_maps, core_ids=list(range(NCORES)), **_cached.get("run_kwargs", {})
    )
    _cached["last_results"] = res

    out = np.zeros((2, S, D), np.float32)
    for c in range(NCORES):
        b = c // 4
        out[b] += res.results[c]["out"]
    out += np.asarray(bo, np.float32)
    return out
